# revision 6
# baseline (speedup 1.0000x reference)
"""Masked multi-head attention (B=4, H=12, S=2048, D=64) on 8 TRN2 NeuronCores.

Returns (out, p_attn) matching the reference:
    scores = (Q @ K^T) / 8, pairwise-masked with mask_q*mask_k (-1e9 fill)
    p_attn = softmax(scores, axis=-1)
    out    = p_attn @ V

Sharding: B*H = 48 (b,h) pairs; core c owns the 6 pairs (b = c//2,
heads (c%2)*6 ..). Each core's pairs share one batch row b, hence one mask.

Mask folding (host side, exact):
    Qp[q, :64] = Q[q, :]*mask_q/8 ; Qp[q, 64] = mask_q
    Kp[k, :64] = K[k, :]         ; Kp[k, 64] = -1e9*(1-mask_k)
    => Qp @ Kp^T = mask_q * (scores + Mcol)   with Mcol[k] in {0, -1e9}
    masked-q rows become exactly 0 -> softmax uniform 1/2048 (= reference).
    masked-k cols get -1e9 + s -> exp underflows to exactly 0 (= reference).

Device (per core, per pair):
  Phase 1 (k-oriented, k compacted to unmasked columns only):
    S^T tile = Kp_c^T-slice  x  Qp^T          [128k, 2048q] PSUM
    E^T = exp(S^T)                            ACT, PSUM->SBUF
    outT_acc[64, 2048q] += V_c-tile^T-matmul  (PE accumulate over k tiles)
  Phase 2 (q-oriented, full k):
    S tile = Qp^T-slice x Kp_full^T           [128q, 2048k] PSUM
    E = exp(S), accum_out -> row sums r       ACT
    P = E * (1/r)                             DVE per-partition scalar
    DMA P -> p_attn rows
Host: out = (outT / r)^T ; masked-q rows of out overwritten with mean(V).
"""

import os
import sys
import time

import numpy as np

if "/opt/trn_rl_repo" not in sys.path:
    sys.path.insert(0, "/opt/trn_rl_repo")

import concourse.bass as bass
import concourse.bacc as bacc
import concourse.bass_utils as bass_utils
import concourse.tile as tile
from concourse import mybir

B, H, S, D = 4, 12, 2048, 64
N_CORES = 8
PAIRS = (B * H) // N_CORES  # 6
HEADS_PER_CORE = PAIRS  # all pairs of a core share one b
KA = D + 1  # augmented contract dim (mask fold)
NQT = S // 128  # 16 q tiles
NEG = np.float32(-1e9)

F32 = mybir.dt.float32
F32R = mybir.dt.float32r

# matmul streaming dtype: "f32r" (1 cyc/row) or "f32" (4 cyc/row, full precision)
MM_DTYPE = os.environ.get("ATTN_MM_DTYPE", "f32r")

LAST_RUN_INFO = {}
_KERNEL_CACHE = {}


MMDT = F32R if MM_DTYPE == "f32r" else F32


def _in_cast(ap):
    return ap.bitcast(MMDT) if MM_DTYPE == "f32r" else ap


def build_kernel(nkt: int):
    """Build the SPMD Bass kernel for nkt compacted k-tiles."""
    nkc = nkt * 128
    nc = bacc.Bacc()

    qpt = nc.dram_tensor("qpt", [PAIRS, KA, S], F32, kind="ExternalInput")
    kptf = nc.dram_tensor("kptf", [PAIRS, KA, S], F32, kind="ExternalInput")
    kptc = nc.dram_tensor("kptc", [PAIRS, KA, nkc], F32, kind="ExternalInput")
    # compacted V, pre-arranged host-side as [128, nkt*64]:
    #   vc[p, t*64 + d] = V[idx[t*128 + p], d]   (zero-padded)
    vc = nc.dram_tensor("vc", [PAIRS, 128, nkt * D], F32, kind="ExternalInput")

    p_attn = nc.dram_tensor("p_attn", [PAIRS, S, S], F32, kind="ExternalOutput")
    outT = nc.dram_tensor("outT", [PAIRS, D, S], F32, kind="ExternalOutput")
    r_out = nc.dram_tensor("r_out", [PAIRS, 128, NQT], F32, kind="ExternalOutput")

    with tile.TileContext(nc) as tc:
        with (
            tc.tile_pool(name="inp", bufs=2) as inp,
            tc.tile_pool(name="ebuf", bufs=3) as ebuf,
            tc.tile_pool(name="otp", bufs=2) as otp,
            tc.tile_pool(name="small", bufs=4) as small,
            tc.tile_pool(name="pst", bufs=2, space="PSUM") as pst,
            tc.tile_pool(name="pout", bufs=1, space="PSUM") as pout,
        ):
            for pair in range(PAIRS):
                # ---- per-pair input loads ----
                qpt_sb = inp.tile([KA, S], MMDT, tag="qpt")
                kptf_sb = inp.tile([KA, S], MMDT, tag="kptf")
                kptc_sb = inp.tile([KA, nkc], MMDT, tag="kptc")
                vc_sb = inp.tile([128, nkt * D], MMDT, tag="vc")
                nc.sync.dma_start(out=qpt_sb, in_=_in_cast(qpt[pair]))
                nc.sync.dma_start(out=kptf_sb, in_=_in_cast(kptf[pair]))
                nc.sync.dma_start(out=kptc_sb, in_=_in_cast(kptc[pair]))
                nc.sync.dma_start(out=vc_sb, in_=_in_cast(vc[pair]))

                # ---- phase 1: k-oriented, compacted k ----
                po = pout.tile([D, S], F32, tag="po")
                for kt in range(nkt):
                    lhs_k = kptc_sb[:, kt * 128 : (kt + 1) * 128]
                    et = ebuf.tile([128, S], MMDT, tag="et")
                    for half in range(2):
                        st = pst.tile([128, 1024], F32, tag="st")
                        for j in range(2):
                            c0 = half * 1024 + j * 512
                            nc.tensor.matmul(
                                st[:, j * 512 : (j + 1) * 512],
                                lhs_k,
                                qpt_sb[:, c0 : c0 + 512],
                                start=True,
                                stop=True,
                            )
                        nc.scalar.activation(
                            out=et[:, half * 1024 : (half + 1) * 1024],
                            in_=st,
                            func=mybir.ActivationFunctionType.Exp,
                        )
                    lhs_v = vc_sb[:, kt * D : (kt + 1) * D]
                    for j in range(4):
                        nc.tensor.matmul(
                            po[:, j * 512 : (j + 1) * 512],
                            lhs_v,
                            et[:, j * 512 : (j + 1) * 512],
                            start=(kt == 0),
                            stop=(kt == nkt - 1),
                        )

                # phase 1 epilogue: PSUM -> SBUF -> DRAM (unnormalized (P@V)^T)
                ot_sb = otp.tile([D, S], F32, tag="ot")
                nc.vector.tensor_copy(ot_sb, po)
                nc.sync.dma_start(out=outT[pair], in_=ot_sb)

                # ---- phase 2: q-oriented, full k ----
                rtile = small.tile([128, NQT], F32, tag="rtile")
                for qt in range(NQT):
                    lhs_q = qpt_sb[:, qt * 128 : (qt + 1) * 128]
                    e = ebuf.tile([128, S], F32, tag="e")
                    racc = small.tile([128, 2], F32, tag="racc")
                    for half in range(2):
                        st = pst.tile([128, 1024], F32, tag="st")
                        for j in range(2):
                            c0 = half * 1024 + j * 512
                            nc.tensor.matmul(
                                st[:, j * 512 : (j + 1) * 512],
                                lhs_q,
                                kptf_sb[:, c0 : c0 + 512],
                                start=True,
                                stop=True,
                            )
                        nc.scalar.activation(
                            out=e[:, half * 1024 : (half + 1) * 1024],
                            in_=st,
                            func=mybir.ActivationFunctionType.Exp,
                            accum_out=racc[:, half : half + 1],
                        )
                    nc.vector.tensor_add(
                        out=rtile[:, qt : qt + 1],
                        in0=racc[:, 0:1],
                        in1=racc[:, 1:2],
                    )
                    rec = small.tile([128, 1], F32, tag="rec")
                    nc.vector.reciprocal(rec, rtile[:, qt : qt + 1])
                    nc.vector.tensor_scalar_mul(out=e, in0=e, scalar1=rec)
                    nc.sync.dma_start(
                        out=p_attn[pair, qt * 128 : (qt + 1) * 128, :], in_=e
                    )
                nc.sync.dma_start(out=r_out[pair], in_=rtile)

    nc.compile()
    return nc


def _prep_core(q, k, v, maskf, b, h0, nkc, idx):
    """Host-side input prep for one core. q/k/v: [H,S,D] slices for batch b."""
    nh = HEADS_PER_CORE
    mq = maskf  # [S] float32, 0/1
    scale = (mq / 8.0).astype(np.float32)

    qpt = np.empty((nh, KA, S), dtype=np.float32)
    kptf = np.empty((nh, KA, S), dtype=np.float32)
    kptc = np.zeros((nh, KA, nkc), dtype=np.float32)
    vc = np.zeros((nh, 128, (nkc // 128) * D), dtype=np.float32)

    mcol = (NEG * (1.0 - mq)).astype(np.float32)  # [S]
    nk = len(idx)
    for j in range(nh):
        hq = q[h0 + j]  # [S, D]
        hk = k[h0 + j]
        hv = v[h0 + j]
        qpt[j, :D, :] = hq.T * scale[None, :]
        qpt[j, D, :] = mq
        kptf[j, :D, :] = hk.T
        kptf[j, D, :] = mcol
        kptc[j, :D, :nk] = hk.T[:, idx]
        kptc[j, D, nk:] = NEG  # pad cols killed via -1e9 * mask_q
        vsel = np.zeros((nkc, D), dtype=np.float32)
        vsel[:nk] = hv[idx]
        # [nkc, D] -> [128, nkt*D] with vc[p, t*D+d] = vsel[t*128+p, d]
        vc[j] = (
            vsel.reshape(nkc // 128, 128, D).transpose(1, 0, 2).reshape(128, -1)
        )
    return {"qpt": qpt, "kptf": kptf, "kptc": kptc, "vc": vc}


def kernel(query, key, value, mask, head_nums=None, _trace=False):
    t_start = time.time()
    q = np.asarray(query, dtype=np.float32)
    k = np.asarray(key, dtype=np.float32)
    v = np.asarray(value, dtype=np.float32)
    m = np.asarray(mask)
    assert q.shape == (B, H, S, D), q.shape

    maskf = m.astype(np.float32)
    idx_by_b = [np.nonzero(m[b])[0] for b in range(B)]
    nk_max = max((len(ix) for ix in idx_by_b), default=0)
    nkc = max(128, ((nk_max + 127) // 128) * 128)
    nkt = nkc // 128

    in_maps = []
    for c in range(N_CORES):
        b = c // 2
        h0 = (c % 2) * HEADS_PER_CORE
        in_maps.append(
            _prep_core(q[b], k[b], v[b], maskf[b], b, h0, nkc, idx_by_b[b])
        )

    key_ = (nkt, MM_DTYPE)
    if key_ not in _KERNEL_CACHE:
        _KERNEL_CACHE[key_] = build_kernel(nkt)
    nc = _KERNEL_CACHE[key_]

    t_prep = time.time()
    res = bass_utils.run_bass_kernel_spmd(
        nc, in_maps, list(range(N_CORES)), trace=_trace
    )
    t_run = time.time()

    LAST_RUN_INFO.clear()
    LAST_RUN_INFO.update(
        exec_time_ns=res.exec_time_ns,
        mean_exec_time_ns=res.mean_exec_time_ns,
        prep_s=t_prep - t_start,
        run_s=t_run - t_prep,
    )

    # ---- host-side assembly ----
    p_attn = np.empty((B, H, S, S), dtype=np.float32)
    out = np.empty((B, H, S, D), dtype=np.float32)
    for c in range(N_CORES):
        b = c // 2
        h0 = (c % 2) * HEADS_PER_CORE
        om = res.results[c]
        p_attn[b, h0 : h0 + HEADS_PER_CORE] = om["p_attn"]
        # r: [pairs, 128, NQT] -> [pairs, S] with q = qt*128 + p
        r = om["r_out"].transpose(0, 2, 1).reshape(PAIRS, S)
        o = om["outT"] / r[:, None, :]  # [pairs, D, S]
        out[b, h0 : h0 + HEADS_PER_CORE] = o.transpose(0, 2, 1)

    # masked-q rows: out = uniform(1/S) @ V = mean of V over k
    for b in range(B):
        mrows = np.nonzero(m[b] == 0)[0]
        if len(mrows):
            vmean = v[b].mean(axis=1)  # [H, D]
            out[b][:, mrows, :] = vmean[:, None, :]

    LAST_RUN_INFO["post_s"] = time.time() - t_run
    return out, p_attn
